# revision 12
# baseline (speedup 1.0000x reference)
"""Trainium2 Bass kernel for nn_CFC_Reformer (CFC + Reformer attention block).

Contract: kernel(**inputs) takes the FULL inputs (x: [8,256,96,96] f32 plus
small conv/attention params), shards x along batch across 8 NeuronCores
(pure data parallel, params replicated), runs one fused Bass/Tile program
per core, and gathers the full [8,128,96,96] f32 output.

Per-core pipeline (one image [256,96,96]):
  A. xr = SiLU(BN(conv3x3(x, w_red)))     -- x fully SBUF-resident, 512-col
     psum windows, 18 fp32r matmuls per window
  B. PSP pooling -> 50 token sums; Reformer bucket argmax/argsort built from
     compare ops + one-hot gather matmuls; kwT = (k_s @ w_q)^T
  C. s = kwT^T @ xr ; e = exp(s) ; ctx2 = (2*v_s)^T @ e * (1/z) broadcast --
     normalization applied to ctx on 128 DVE lanes, not to e on 8
  D. y = SiLU(BN((w_la1/2) @ ctx2)) once + Pool-engine shifted replicas;
     msk = sigmoid(conv3x3(y, 2*w_la2)); out = ctx2*msk + xr
     (uses 1 + tanh(m) == 2*sigmoid(2m), scale factors folded into weights)

Spatial layout on chip: width padded 96 -> 98 (one zero col each side) so
3x3 conv taps are pure column offsets into the flattened row-major image.
"""

import numpy as np
import ml_dtypes

import concourse.bass as bass
import concourse.bass_isa as bass_isa
import concourse.bacc as bacc
import concourse.mybir as mybir
import concourse.tile as tile
from concourse.bass_utils import run_bass_kernel_spmd

F32 = mybir.dt.float32
F32R = mybir.dt.float32r
BF16 = mybir.dt.bfloat16
AF = mybir.ActivationFunctionType
ALU = mybir.AluOpType
AX = mybir.AxisListType

# Problem shapes (hardcoded per the harness contract).
B, H, W = 8, 96, 96
CIN, COUT, QD, NH = 256, 128, 32, 8
LA_MID = 16
EPS = 1e-5
WP = 98                     # padded row width (1 zero col each side)
NPIX = H * WP               # 9408 padded pixels
WIN = 512                   # phase A / C1 psum window (max moving free dim)
NWIN = (NPIX + WIN - 1) // WIN          # 19 windows (18x512 + 1x192)
TCOLS = 4 * WP              # row-aligned tile for phases C2/D
NT = NPIX // TCOLS          # 24 spatial tiles
XTOT = 1 + 98 * WP + 8      # host-padded x: guard col + 98 padded rows + slack
Y_G = WP + 2                # y top guard (one padded row + margin)
Y_SZ = Y_G + NPIX + Y_G + 6 # y tile with top/bottom zero guards
NTOK = 50                   # 36 + 9 + 4 + 1 PSP tokens
BIGF = 1.0e5

_BUILD_CACHE = {}


def _round_fp32r(a):
    """Round-to-nearest-even to fp32r (e8m13) so PE truncation is exact."""
    u = np.ascontiguousarray(a, np.float32).view(np.uint32)
    r = (u + 0x1FF + ((u >> 10) & 1)) & np.uint32(0xFFFFFC00)
    return r.view(np.float32)


def _bf16(a):
    return np.ascontiguousarray(np.asarray(a, np.float32)).astype(
        ml_dtypes.bfloat16)


def _host_prep(inp):
    """Fold BN into conv weights and lay every parameter out exactly as the
    SBUF tiles expect ([partition, free], contraction on partitions)."""
    f = np.float32
    w_red = np.asarray(inp["w_red"], f)
    binv = np.asarray(inp["bng"], f) / np.sqrt(np.asarray(inp["bnv"], f) + EPS)
    bnbias = np.asarray(inp["bnb"], f) - np.asarray(inp["bnm"], f) * binv
    wf = w_red * binv[:, None, None, None]          # [COUT, CIN, 3, 3]

    w1t = np.empty((128, 2304), f)
    for kc in range(2):
        for dy in range(3):
            for dx in range(3):
                t = kc * 9 + dy * 3 + dx
                # [ci_local, co]
                w1t[:, t * 128:(t + 1) * 128] = wf[:, kc * 128:(kc + 1) * 128, dy, dx].T

    w_la2 = np.asarray(inp["w_la2"], f)             # [COUT, LA_MID, 3, 3]
    # K-packed: partitions (dx-shift s)*32 + ci (32-stride for legal engine
    # partition starts; odd half zero), one matmul per dy tap. 2x scale folds
    # the tanh->sigmoid identity: 1 + tanh(m) = 2*sigmoid(2m).
    wla2t = np.zeros((96, 3 * 128), f)
    for dy in range(3):
        for sft in range(3):
            wla2t[sft * 32:sft * 32 + LA_MID, dy * 128:(dy + 1) * 128] = \
                2.0 * w_la2[:, :, dy, sft].T

    lasc = np.asarray(inp["lag"], f) / np.sqrt(np.asarray(inp["lav"], f) + EPS)
    labi = np.asarray(inp["lab"], f) - np.asarray(inp["lam"], f) * lasc

    inv_area = np.concatenate([
        np.full(36, 1.0 / 256), np.full(9, 1.0 / 1024),
        np.full(4, 1.0 / 2304), np.full(1, 1.0 / 9216)]).astype(f)

    jlt = np.tril(np.ones((NH, NH), f), k=-1)        # jlt[i,j] = 1 if j < i
    p_iota = np.tile(np.arange(NH, dtype=f), (NH, 1))

    lsh = np.asarray(inp["lsh"], f)
    b_k = np.asarray(inp["b_k"], f)

    return {
        "w1t": _round_fp32r(w1t),
        "bnbias": np.ascontiguousarray(bnbias.reshape(128, 1)),
        "wkT": np.ascontiguousarray(np.asarray(inp["w_k"], f).T),      # [128,32]
        # 2x: ctx is produced pre-doubled (ctx2 = 2*ctx) for the sigmoid trick
        "wvT": np.ascontiguousarray(2.0 * np.asarray(inp["w_v"], f).T),
        "wq": np.ascontiguousarray(np.asarray(inp["w_q"], f)),         # [32,128]
        "bq": np.ascontiguousarray(np.asarray(inp["b_q"], f).reshape(QD, 1)),
        "lshT": np.ascontiguousarray(lsh.T),                           # [32,8]
        "lshbk": np.ascontiguousarray((lsh @ b_k).reshape(NH, 1)),
        "bk8": np.ascontiguousarray(np.tile(b_k, (NH, 1))),            # [8,32]
        "bv8": np.ascontiguousarray(
            2.0 * np.tile(np.asarray(inp["b_v"], f), (NH, 1))),
        # 0.5x: y-path consumes ctx2 but must see plain ctx
        "wla1T": _bf16(0.5 * np.asarray(inp["w_la1"], f).T),           # [128,16]
        "lasc": np.ascontiguousarray(lasc.reshape(LA_MID, 1)),
        "labi": np.ascontiguousarray(labi.reshape(LA_MID, 1)),
        "wla2t": _bf16(wla2t),
        "invarea": np.ascontiguousarray(np.tile(inv_area, (NH, 1))),   # [8,50]
        "iota50": np.ascontiguousarray(
            np.tile(np.arange(NTOK, dtype=f), (NH, 1))),               # [8,50]
        "iotapb": np.ascontiguousarray(
            np.tile(np.arange(NTOK, dtype=f) + BIGF, (NH, 1))),
        "jlt": jlt,
        "piota": p_iota,
        "ones18": np.ones((1, NH), f),
        "ident": np.eye(128, dtype=f),
    }


# (name, shape, sbuf dtype); dram dtype is f32 unless the sbuf tile is BF16,
# F32R tiles are bitcast views of f32 dram tensors.
PARAMS = [
    ("w1t", (128, 2304), F32R), ("bnbias", (128, 1), F32),
    ("wkT", (128, QD), F32), ("wvT", (128, 128), F32),
    ("wq", (QD, 128), F32), ("bq", (QD, 1), F32),
    ("lshT", (QD, NH), F32), ("lshbk", (NH, 1), F32),
    ("bk8", (NH, QD), F32), ("bv8", (NH, 128), F32),
    ("wla1T", (128, LA_MID), BF16), ("lasc", (LA_MID, 1), F32),
    ("labi", (LA_MID, 1), F32), ("wla2t", (96, 3 * 128), BF16),
    ("invarea", (NH, NTOK), F32), ("iota50", (NH, NTOK), F32),
    ("iotapb", (NH, NTOK), F32),
    ("jlt", (NH, NH), F32), ("piota", (NH, NH), F32),
    ("ones18", (1, NH), F32),
    ("ident", (128, 128), F32),
]

# x load chunks (cols of the padded image): first small chunk lets the PE
# start early, later chunks stream in under compute.
XCHUNKS = [(0, 1200), (1200, 3600), (3600, 6600), (6600, XTOT)]


def build_program(reps=1):
    """Build the single-core SPMD Bass/Tile program. Same program runs on all
    8 cores; only the 'x' input differs per core. reps>1 repeats the whole
    compute pipeline (timing variant: slope over reps isolates per-iter
    device time from dispatch overhead)."""
    nc = bacc.Bacc("TRN2", target_bir_lowering=False, debug=False)

    di = {}
    di["x"] = nc.dram_tensor("x", [CIN, XTOT], F32, kind="ExternalInput").ap()
    for name, shape, dt_ in PARAMS:
        dram_dt = BF16 if dt_ is BF16 else F32
        di[name] = nc.dram_tensor(name, list(shape), dram_dt,
                                  kind="ExternalInput").ap()
    out_d = nc.dram_tensor("out", [COUT, NPIX], F32, kind="ExternalOutput").ap()

    with tile.TileContext(nc) as tc:
      # one long-lived pool holds every persistent tile (unique tag = own slot)
      with tc.tile_pool(name="perm", bufs=1) as perm:
        def ptile(name, shape, dt=F32):
            return perm.tile(list(shape), dt, name=name, tag=name)

        xk = [ptile(f"xk{kc}", [128, XTOT], F32R) for kc in range(2)]
        xr = ptile("xr", [128, NPIX], F32R)
        ctx_s = ptile("ctx_s", [128, NPIX], BF16)
        y_s = ptile("y_s", [96, Y_SZ], BF16)
        rowsum6 = ptile("rowsum6", [128, 576])
        S_s = ptile("S_s", [128, 64])

        sb = {}
        # critical-path first: conv weights, then the first x chunks, then
        # everything else
        sb["w1t"] = ptile("sb_w1t", [128, 2304], F32R)
        nc.sync.dma_start(out=sb["w1t"][:, :], in_=di["w1t"][:, :].bitcast(F32R))
        c0, c1 = XCHUNKS[0]
        for kc in range(2):
            nc.sync.dma_start(
                out=xk[kc][:, c0:c1],
                in_=di["x"][kc * 128:(kc + 1) * 128, c0:c1].bitcast(F32R))
        for name, shape, dt_ in PARAMS:
            if name == "w1t":
                continue
            sb[name] = ptile("sb_" + name, list(shape), dt_)
            src = di[name][:, :]
            if dt_ is F32R:
                src = src.bitcast(F32R)
            nc.sync.dma_start(out=sb[name][:, :], in_=src)
        for c0, c1 in XCHUNKS[1:]:
            for kc in range(2):
                nc.sync.dma_start(
                    out=xk[kc][:, c0:c1],
                    in_=di["x"][kc * 128:(kc + 1) * 128, c0:c1].bitcast(F32R))

        # one-time zero of the whole y tile: the 32-stride packing leaves
        # unused partitions whose wla2t rows are zero, but 0 * garbage-inf
        # would poison psum (engine APs need 32-aligned partition starts,
        # so zero everything once; per-rep guards are re-zeroed below)
        nc.gpsimd.memset(y_s[:, :], 0.0)

        for _rep in range(reps):
            # ============ Phase A: conv3x3 + BN + SiLU -> xr ==============
            nred = 0
            with tc.tile_pool(name="apsum", bufs=4, space="PSUM") as apool:
                for w in range(NWIN):
                    c0 = w * WIN
                    L = min(WIN, NPIX - c0)
                    ps = apool.tile([128, WIN], F32, tag="apsum")
                    for kc in range(2):
                        for dy in range(3):
                            for dx in range(3):
                                t = kc * 9 + dy * 3 + dx
                                off = c0 + dy * WP + dx
                                nc.tensor.matmul(
                                    ps[:, 0:L],
                                    sb["w1t"][:, t * 128:(t + 1) * 128],
                                    xk[kc][:, off:off + L],
                                    start=(t == 0), stop=(t == 17))
                    nc.scalar.activation(
                        xr[:, c0:c0 + L], ps[:, 0:L], AF.Silu,
                        bias=sb["bnbias"][:, 0:1])
                    # PSP stage 1 on finished 24-row chunks (rows r need cols
                    # through (r+1)*WP <= c0+L)
                    while nred < 4 and (nred + 1) * 24 * WP <= c0 + L:
                        r0 = nred * 24
                        xrb = xr[:, r0 * WP:(r0 + 24) * WP].rearrange(
                            "p (y c) -> p y c", c=WP)[:, :, 1:1 + W].rearrange(
                            "p y (j u) -> p y j u", u=16)
                        nc.vector.tensor_reduce(
                            rowsum6[:, nred * 144:(nred + 1) * 144].rearrange(
                                "p (y j) -> p y j", j=6),
                            xrb, axis=AX.X, op=ALU.add)
                        nred += 1

            # ============ Phase B: tokens + reformer gather ===============
            # S6 [128,36]: column sums of rowsum6 over 16-row groups
            nc.vector.tensor_reduce(
                S_s[:, 0:36].rearrange("p (i j) -> p i j", j=6),
                rowsum6[:, :].rearrange("p (i u j) -> p i j u", i=6, u=16, j=6),
                axis=AX.X, op=ALU.add)
            s3t = ptile("s3t", [128, 18])
            nc.vector.tensor_reduce(
                s3t[:, :].rearrange("p (i a j) -> p i a j", i=3, a=2, j=3),
                S_s[:, 0:36].rearrange("p (i a j b) -> p i a j b", i=3, a=2, j=3, b=2),
                axis=AX.X, op=ALU.add)
            nc.vector.tensor_reduce(
                S_s[:, 36:45].rearrange("p (i j) -> p i j", j=3),
                s3t[:, :].rearrange("p (i a j) -> p i j a", i=3, a=2, j=3),
                axis=AX.X, op=ALU.add)
            s2t = ptile("s2t", [128, 12])
            nc.vector.tensor_reduce(
                s2t[:, :].rearrange("p (i a j) -> p i a j", i=2, a=3, j=2),
                S_s[:, 0:36].rearrange("p (i a j b) -> p i a j b", i=2, a=3, j=2, b=3),
                axis=AX.X, op=ALU.add)
            nc.vector.tensor_reduce(
                S_s[:, 45:49].rearrange("p (i j) -> p i j", j=2),
                s2t[:, :].rearrange("p (i a j) -> p i j a", i=2, a=3, j=2),
                axis=AX.X, op=ALU.add)
            nc.vector.tensor_reduce(
                S_s[:, 49:50], S_s[:, 0:36].rearrange("p (i j) -> p i j", j=6),
                axis=AX.XY, op=ALU.add)

            kS = ptile("kS", [QD, NTOK])
            vS = ptile("vS", [128, NTOK])
            ktok8 = ptile("ktok8", [NH, QD])
            vtok8 = ptile("vtok8", [NH, 128])
            Lsc = ptile("Lsc", [NH, NTOK])
            eqt = ptile("eqt", [NH, NTOK])
            t1 = ptile("t1", [NH, NTOK])
            maxv = ptile("maxv", [NH, 1])
            bmin = ptile("bmin", [NH, 1])
            bT = ptile("bT", [1, NH])
            ranksrc = ptile("ranksrc", [NH, NH])
            eqm = ptile("eqm", [NH, NH])
            rank = ptile("rank", [NH, 1])
            onehot = ptile("onehot", [NH, NH])
            ksT = ptile("ksT", [QD, NH])
            vs_g = ptile("vs_g", [NH, 128], BF16)
            kwT = ptile("kwT", [128, NH], F32R)
            sbias = ptile("sbias", [NH, 1])

            with tc.tile_pool(name="bpsum", bufs=2, space="PSUM") as bpool:
                kS_p = bpool.tile([QD, NTOK], F32, tag="b1")
                nc.tensor.matmul(kS_p[:, :], sb["wkT"][:, :], S_s[:, 0:NTOK],
                                 start=True, stop=True)
                nc.scalar.copy(kS[:, :], kS_p[:, :])
                vS_p = bpool.tile([128, NTOK], F32, tag="b2")
                nc.tensor.matmul(vS_p[:, :], sb["wvT"][:, :], S_s[:, 0:NTOK],
                                 start=True, stop=True)
                nc.scalar.copy(vS[:, :], vS_p[:, :])

                # bucket logits over all 50 tokens (area-normalized + lsh@b_k)
                L_p = bpool.tile([NH, NTOK], F32, tag="b1")
                nc.tensor.matmul(L_p[:, :], sb["lshT"][:, :], kS[:, :],
                                 start=True, stop=True)
                nc.vector.tensor_tensor(Lsc[:, :], L_p[:, :], sb["invarea"][:, :],
                                        op=ALU.mult)
                nc.vector.tensor_scalar_add(Lsc[:, :], Lsc[:, :], sb["lshbk"][:, 0:1])
                # argmax over tokens (first occurrence)
                nc.vector.tensor_reduce(maxv[:, :], Lsc[:, :], axis=AX.X, op=ALU.max)
                nc.vector.tensor_scalar(eqt[:, :], Lsc[:, :], maxv[:, 0:1], None,
                                        op0=ALU.is_equal)
                # t1 = (eqt * -BIG) + (iota + BIG): iota where eq, BIG elsewhere
                nc.vector.scalar_tensor_tensor(
                    t1[:, :], eqt[:, :], -BIGF, sb["iotapb"][:, :],
                    op0=ALU.mult, op1=ALU.add)
                nc.vector.tensor_reduce(bmin[:, :], t1[:, :], axis=AX.X, op=ALU.min)

                # stable argsort rank of the 8 bucket ids
                bT_p = bpool.tile([1, NH], F32, tag="b1")
                nc.tensor.matmul(bT_p[:, :], bmin[:, :], sb["ident"][0:NH, 0:NH],
                                 start=True, stop=True)
                nc.scalar.copy(bT[:, :], bT_p[:, :])
                Bij_p = bpool.tile([NH, NH], F32, tag="b2")
                nc.tensor.matmul(Bij_p[:, :], sb["ones18"][:, :], bT[:, :],
                                 start=True, stop=True)
                nc.vector.tensor_scalar(ranksrc[:, :], Bij_p[:, :], bmin[:, 0:1], None,
                                        op0=ALU.is_lt)
                nc.vector.tensor_scalar(eqm[:, :], Bij_p[:, :], bmin[:, 0:1], None,
                                        op0=ALU.is_equal)
                nc.vector.tensor_tensor(eqm[:, :], eqm[:, :], sb["jlt"][:, :],
                                        op=ALU.mult)
                nc.vector.tensor_tensor(ranksrc[:, :], ranksrc[:, :], eqm[:, :],
                                        op=ALU.add)
                nc.vector.tensor_reduce(rank[:, :], ranksrc[:, :], axis=AX.X, op=ALU.add)
                nc.vector.tensor_scalar(onehot[:, :], sb["piota"][:, :], rank[:, 0:1],
                                        None, op0=ALU.is_equal)

                # first-8 tokens to [token, feat] layout (+ mean scale & bias)
                kt_p = bpool.tile([NH, QD], F32, tag="b1")
                nc.tensor.transpose(kt_p[:, :], kS[:, 0:NH], sb["ident"][0:QD, 0:QD])
                nc.vector.tensor_scalar(ktok8[:, :], kt_p[:, :], 1.0 / 256, None,
                                        op0=ALU.mult)
                nc.vector.tensor_tensor(ktok8[:, :], ktok8[:, :], sb["bk8"][:, :],
                                        op=ALU.add)
                vt_p = bpool.tile([NH, 128], F32, tag="b2")
                nc.tensor.transpose(vt_p[:, :], vS[:, 0:NH], sb["ident"][:, :])
                nc.vector.tensor_scalar(vtok8[:, :], vt_p[:, :], 1.0 / 256, None,
                                        op0=ALU.mult)
                nc.vector.tensor_tensor(vtok8[:, :], vtok8[:, :], sb["bv8"][:, :],
                                        op=ALU.add)

                # gather sorted tokens, fold w_q, score bias
                ksT_p = bpool.tile([QD, NH], F32, tag="b1")
                nc.tensor.matmul(ksT_p[:, :], ktok8[:, :], onehot[:, :],
                                 start=True, stop=True)
                nc.scalar.copy(ksT[:, :], ksT_p[:, :])
                vs_p = bpool.tile([NH, 128], F32, tag="b2")
                nc.tensor.matmul(vs_p[:, :], onehot[:, :], vtok8[:, :],
                                 start=True, stop=True)
                nc.scalar.copy(vs_g[:, :], vs_p[:, :])
                kw_p = bpool.tile([128, NH], F32, tag="b1")
                nc.tensor.matmul(kw_p[:, :], sb["wq"][:, :], ksT[:, :],
                                 start=True, stop=True)
                nc.scalar.copy(kwT[:, :], kw_p[:, :])
                sb_p = bpool.tile([NH, 1], F32, tag="b2")
                nc.tensor.matmul(sb_p[:, :], ksT[:, :], sb["bq"][:, :],
                                 start=True, stop=True)
                nc.scalar.copy(sbias[:, :], sb_p[:, :])

            # ======== Phase C1: attention (ACT runs exp only) =============
            with (
                tc.tile_pool(name="cpool", bufs=3) as cpool,
                tc.tile_pool(name="cps_s", bufs=2, space="PSUM") as ps_s,
                tc.tile_pool(name="cps_ctx", bufs=2, space="PSUM") as ps_ctx,
            ):
                for w in range(NWIN):
                    c0 = w * WIN
                    L = min(WIN, NPIX - c0)
                    s_p = ps_s.tile([NH, WIN], F32, tag="s")
                    nc.tensor.matmul(s_p[:, 0:L], kwT[:, :], xr[:, c0:c0 + L],
                                     start=True, stop=True)
                    e_t = cpool.tile([NH, WIN], BF16, tag="e")
                    nc.scalar.activation(e_t[:, 0:L], s_p[:, 0:L], AF.Exp,
                                         bias=sbias[:, 0:1])
                    # z on the idle Pool engine (broadcast across the 8
                    # partitions), so recip + e-scale run on SBUF bf16 where
                    # DVE gets its 2x/4x packed modes
                    z_t = cpool.tile([NH, WIN], BF16, tag="z")
                    with nc.allow_low_precision(reason="bf16 softmax scale"):
                        nc.gpsimd.partition_all_reduce(
                            z_t[:, 0:L], e_t[:, 0:L], channels=NH,
                            reduce_op=bass_isa.ReduceOp.add)
                        rc_t = cpool.tile([NH, WIN], BF16, tag="rc")
                        nc.vector.reciprocal(rc_t[:, 0:L], z_t[:, 0:L])
                        e2_t = cpool.tile([NH, WIN], BF16, tag="e2")
                        nc.vector.tensor_tensor(e2_t[:, 0:L], e_t[:, 0:L],
                                                rc_t[:, 0:L], op=ALU.mult)
                    ctx_p = ps_ctx.tile([128, WIN], F32, tag="ctx")
                    nc.tensor.matmul(ctx_p[:, 0:L], vs_g[:, :], e2_t[:, 0:L],
                                     start=True, stop=True)
                    # psum -> SBUF move alternates ACT/DVE to balance load
                    if w % 2 == 0:
                        nc.scalar.copy(ctx_s[:, c0:c0 + L], ctx_p[:, 0:L])
                    else:
                        with nc.allow_low_precision(reason="bf16 ctx"):
                            nc.vector.tensor_copy(ctx_s[:, c0:c0 + L],
                                                  ctx_p[:, 0:L])

            # keep exp and silu in separate contiguous ACT runs (LUT reloads)
            tc.no_sync_barrier()

            # ==== Phase C2 (silu -> y group1 + Pool replicas) + Phase D ====
            def emit_y(t):
                c0 = t * TCOLS
                yp_p = ps_yp.tile([LA_MID, TCOLS], F32, tag="yp")
                nc.tensor.matmul(yp_p[:, :], sb["wla1T"][:, :],
                                 ctx_s[:, c0:c0 + TCOLS], start=True, stop=True)
                pv = yp_p[:, :].rearrange("p (r c) -> p r c", c=WP)[:, :, 1:1 + W]
                base = Y_G + c0
                dst1 = y_s[32:32 + LA_MID, base:base + TCOLS].rearrange(
                    "p (r c) -> p r c", c=WP)[:, :, 1:1 + W]
                nc.scalar.activation(dst1, pv, AF.Silu,
                                     bias=sb["labi"][:, 0:1],
                                     scale=sb["lasc"][:, 0:1])
                # shifted replicas for the dx taps: group0 = +1 col, group2 = -1
                # (split across the idle Pool engine and DVE's 2x bf16 path)
                for g, p0, eng in ((0, 0, nc.gpsimd), (2, 64, nc.vector)):
                    dstg = y_s[p0:p0 + LA_MID,
                               base + (1 - g):base + (1 - g) + TCOLS].rearrange(
                        "p (r c) -> p r c", c=WP)[:, :, 1:1 + W]
                    eng.tensor_copy(dstg, dst1)

            DG = 3      # tiles per consolidated DVE/DMA group
            grp = {}

            def emit_mask(t):
                c0 = t * TCOLS
                m_p = dps.tile([128, TCOLS], F32, tag="m")
                for dy in range(3):
                    off = Y_G + c0 + (dy - 1) * WP
                    nc.tensor.matmul(
                        m_p[:, :],
                        sb["wla2t"][:, dy * 128:(dy + 1) * 128],
                        y_s[:, off:off + TCOLS],
                        start=(dy == 0), stop=(dy == 2))
                gi = t % DG
                if gi == 0:
                    grp["msk"] = dpool.tile([128, DG * TCOLS], BF16, name="mskg", tag="msk")
                nc.scalar.activation(grp["msk"][:, gi * TCOLS:(gi + 1) * TCOLS],
                                     m_p[:, :], AF.Sigmoid)
                if gi == DG - 1:
                    g0 = (t - DG + 1) * TCOLS
                    GL = DG * TCOLS
                    ct = dpool.tile([128, GL], BF16, tag="ct")
                    with nc.allow_low_precision(reason="bf16 mask product"):
                        nc.vector.tensor_tensor(ct[:, :], ctx_s[:, g0:g0 + GL],
                                                grp["msk"][:, :], op=ALU.mult)
                    o_t = dpool.tile([128, GL], F32, tag="o")
                    nc.vector.tensor_tensor(o_t[:, :], ct[:, :],
                                            xr[:, g0:g0 + GL], op=ALU.add)
                    nc.sync.dma_start(out=out_d[:, g0:g0 + GL], in_=o_t[:, :])

            with (
                tc.tile_pool(name="dpool", bufs=2) as dpool,
                tc.tile_pool(name="cps_yp", bufs=2, space="PSUM") as ps_yp,
                tc.tile_pool(name="dpsum", bufs=2, space="PSUM") as dps,
            ):
                # two contiguous ACT runs (silu table, then sigmoid table):
                # interleaving emit_y/emit_mask would reload the LUT per tile
                for t in range(NT):
                    emit_y(t)
                for t in range(NT):
                    emit_mask(t)

    nc.compile()
    return nc


def get_program(reps=1):
    key = ("nc", reps)
    if key not in _BUILD_CACHE:
        _BUILD_CACHE[key] = build_program(reps)
    return _BUILD_CACHE[key]


def pad_x(xb):
    """[256,96,96] -> zero-framed [256, XTOT] fp32r-rounded."""
    xp = np.zeros((CIN, XTOT), np.float32)
    body = np.zeros((CIN, 98, WP), np.float32)
    body[:, 1:1 + H, 1:1 + W] = xb
    xp[:, 1:1 + 98 * WP] = body.reshape(CIN, 98 * WP)
    return _round_fp32r(xp)


def unpad_out(flat):
    """[128, 9408] padded rows -> [128, 96, 96]."""
    return np.ascontiguousarray(flat.reshape(COUT, H, WP)[:, :, 1:1 + W])


def kernel(**inputs):
    x = np.ascontiguousarray(np.asarray(inputs["x"], np.float32))
    assert x.shape == (B, CIN, H, W)
    weights = _host_prep(inputs)
    nc = get_program()
    in_maps = [dict(weights, x=pad_x(x[b])) for b in range(B)]
    res = run_bass_kernel_spmd(nc, in_maps, list(range(B)))
    out = np.stack([unpad_out(res.results[b]["out"]) for b in range(B)], axis=0)
    return out.astype(np.float32)


# revision 13
# speedup vs baseline: 41.5689x; 41.5689x over previous
"""Trainium2 Bass kernel for nn_CFC_Reformer (CFC + Reformer attention block).

Contract: kernel(**inputs) takes the FULL inputs (x: [8,256,96,96] f32 plus
small conv/attention params), shards x along batch across 8 NeuronCores
(pure data parallel, params replicated), runs one fused Bass/Tile program
per core, and gathers the full [8,128,96,96] f32 output.

Per-core pipeline (one image [256,96,96]):
  A. xr = SiLU(BN(conv3x3(x, w_red)))     -- x fully SBUF-resident, 512-col
     psum windows, 18 fp32r matmuls per window
  B. PSP pooling -> 50 token sums; Reformer bucket argmax/argsort built from
     compare ops + one-hot gather matmuls; kwT = (k_s @ w_q)^T
  C. s = kwT^T @ xr ; e = exp(s) ; ctx2 = (2*v_s)^T @ e * (1/z) broadcast --
     normalization applied to ctx on 128 DVE lanes, not to e on 8
  D. y = SiLU(BN((w_la1/2) @ ctx2)) once + Pool-engine shifted replicas;
     msk = sigmoid(conv3x3(y, 2*w_la2)); out = ctx2*msk + xr
     (uses 1 + tanh(m) == 2*sigmoid(2m), scale factors folded into weights)

Spatial layout on chip: width padded 96 -> 98 (one zero col each side) so
3x3 conv taps are pure column offsets into the flattened row-major image.
"""

import numpy as np
import ml_dtypes

import concourse.bass as bass
import concourse.bass_isa as bass_isa
import concourse.bacc as bacc
import concourse.mybir as mybir
import concourse.tile as tile
from concourse.bass_utils import run_bass_kernel_spmd

F32 = mybir.dt.float32
F32R = mybir.dt.float32r
BF16 = mybir.dt.bfloat16
AF = mybir.ActivationFunctionType
ALU = mybir.AluOpType
AX = mybir.AxisListType

# Problem shapes (hardcoded per the harness contract).
B, H, W = 8, 96, 96
CIN, COUT, QD, NH = 256, 128, 32, 8
LA_MID = 16
EPS = 1e-5
WP = 98                     # padded row width (1 zero col each side)
NPIX = H * WP               # 9408 padded pixels
WIN = 512                   # phase A / C1 psum window (max moving free dim)
NWIN = (NPIX + WIN - 1) // WIN          # 19 windows (18x512 + 1x192)
TCOLS = 4 * WP              # row-aligned tile for phases C2/D
NT = NPIX // TCOLS          # 24 spatial tiles
XTOT = 1 + 98 * WP + 8      # host-padded x: guard col + 98 padded rows + slack
Y_G = WP + 2                # y top guard (one padded row + margin)
Y_SZ = Y_G + NPIX + Y_G + 6 # y tile with top/bottom zero guards
NTOK = 50                   # 36 + 9 + 4 + 1 PSP tokens
BIGF = 1.0e5

_BUILD_CACHE = {}


def _round_fp32r(a):
    """Round-to-nearest-even to fp32r (e8m13) so PE truncation is exact."""
    u = np.ascontiguousarray(a, np.float32).view(np.uint32)
    r = (u + 0x1FF + ((u >> 10) & 1)) & np.uint32(0xFFFFFC00)
    return r.view(np.float32)


def _bf16(a):
    return np.ascontiguousarray(np.asarray(a, np.float32)).astype(
        ml_dtypes.bfloat16)


def _host_prep(inp):
    """Fold BN into conv weights and lay every parameter out exactly as the
    SBUF tiles expect ([partition, free], contraction on partitions)."""
    f = np.float32
    w_red = np.asarray(inp["w_red"], f)
    binv = np.asarray(inp["bng"], f) / np.sqrt(np.asarray(inp["bnv"], f) + EPS)
    bnbias = np.asarray(inp["bnb"], f) - np.asarray(inp["bnm"], f) * binv
    wf = w_red * binv[:, None, None, None]          # [COUT, CIN, 3, 3]

    w1t = np.empty((128, 2304), f)
    for kc in range(2):
        for dy in range(3):
            for dx in range(3):
                t = kc * 9 + dy * 3 + dx
                # [ci_local, co]
                w1t[:, t * 128:(t + 1) * 128] = wf[:, kc * 128:(kc + 1) * 128, dy, dx].T

    w_la2 = np.asarray(inp["w_la2"], f)             # [COUT, LA_MID, 3, 3]
    # K-packed: partitions (dx-shift s)*32 + ci (32-stride for legal engine
    # partition starts; odd half zero), one matmul per dy tap. 2x scale folds
    # the tanh->sigmoid identity: 1 + tanh(m) = 2*sigmoid(2m).
    wla2t = np.zeros((96, 3 * 128), f)
    for dy in range(3):
        for sft in range(3):
            wla2t[sft * 32:sft * 32 + LA_MID, dy * 128:(dy + 1) * 128] = \
                2.0 * w_la2[:, :, dy, sft].T

    lasc = np.asarray(inp["lag"], f) / np.sqrt(np.asarray(inp["lav"], f) + EPS)
    labi = np.asarray(inp["lab"], f) - np.asarray(inp["lam"], f) * lasc

    inv_area = np.concatenate([
        np.full(36, 1.0 / 256), np.full(9, 1.0 / 1024),
        np.full(4, 1.0 / 2304), np.full(1, 1.0 / 9216)]).astype(f)

    jlt = np.tril(np.ones((NH, NH), f), k=-1)        # jlt[i,j] = 1 if j < i
    p_iota = np.tile(np.arange(NH, dtype=f), (NH, 1))

    lsh = np.asarray(inp["lsh"], f)
    b_k = np.asarray(inp["b_k"], f)

    return {
        "w1t": _round_fp32r(w1t),
        "bnbias": np.ascontiguousarray(bnbias.reshape(128, 1)),
        "wkT": np.ascontiguousarray(np.asarray(inp["w_k"], f).T),      # [128,32]
        # 2x: ctx is produced pre-doubled (ctx2 = 2*ctx) for the sigmoid trick
        "wvT": np.ascontiguousarray(2.0 * np.asarray(inp["w_v"], f).T),
        "wq": np.ascontiguousarray(np.asarray(inp["w_q"], f)),         # [32,128]
        "bq": np.ascontiguousarray(np.asarray(inp["b_q"], f).reshape(QD, 1)),
        "lshT": np.ascontiguousarray(lsh.T),                           # [32,8]
        "lshbk": np.ascontiguousarray((lsh @ b_k).reshape(NH, 1)),
        "bk8": np.ascontiguousarray(np.tile(b_k, (NH, 1))),            # [8,32]
        "bv8": np.ascontiguousarray(
            2.0 * np.tile(np.asarray(inp["b_v"], f), (NH, 1))),
        # 0.5x: y-path consumes ctx2 but must see plain ctx
        "wla1T": _bf16(0.5 * np.asarray(inp["w_la1"], f).T),           # [128,16]
        "lasc": np.ascontiguousarray(lasc.reshape(LA_MID, 1)),
        "labi": np.ascontiguousarray(labi.reshape(LA_MID, 1)),
        "wla2t": _bf16(wla2t),
        "invarea": np.ascontiguousarray(np.tile(inv_area, (NH, 1))),   # [8,50]
        "iota50": np.ascontiguousarray(
            np.tile(np.arange(NTOK, dtype=f), (NH, 1))),               # [8,50]
        "iotapb": np.ascontiguousarray(
            np.tile(np.arange(NTOK, dtype=f) + BIGF, (NH, 1))),
        "ones88b": _bf16(np.ones((NH, NH), f)),
        "jlt": jlt,
        "piota": p_iota,
        "ones18": np.ones((1, NH), f),
        "ident": np.eye(128, dtype=f),
    }


# (name, shape, sbuf dtype); dram dtype is f32 unless the sbuf tile is BF16,
# F32R tiles are bitcast views of f32 dram tensors.
PARAMS = [
    ("w1t", (128, 2304), F32R), ("bnbias", (128, 1), F32),
    ("wkT", (128, QD), F32), ("wvT", (128, 128), F32),
    ("wq", (QD, 128), F32), ("bq", (QD, 1), F32),
    ("lshT", (QD, NH), F32), ("lshbk", (NH, 1), F32),
    ("bk8", (NH, QD), F32), ("bv8", (NH, 128), F32),
    ("wla1T", (128, LA_MID), BF16), ("lasc", (LA_MID, 1), F32),
    ("labi", (LA_MID, 1), F32), ("wla2t", (96, 3 * 128), BF16),
    ("invarea", (NH, NTOK), F32), ("iota50", (NH, NTOK), F32),
    ("iotapb", (NH, NTOK), F32),
    ("jlt", (NH, NH), F32), ("piota", (NH, NH), F32),
    ("ones88b", (NH, NH), BF16), ("ones18", (1, NH), F32),
    ("ident", (128, 128), F32),
]

# x load chunks (cols of the padded image): first small chunk lets the PE
# start early, later chunks stream in under compute.
XCHUNKS = [(0, 1200), (1200, 3600), (3600, 6600), (6600, XTOT)]


def build_program(reps=1):
    """Build the single-core SPMD Bass/Tile program. Same program runs on all
    8 cores; only the 'x' input differs per core. reps>1 repeats the whole
    compute pipeline (timing variant: slope over reps isolates per-iter
    device time from dispatch overhead)."""
    nc = bacc.Bacc("TRN2", target_bir_lowering=False, debug=False)

    di = {}
    di["x"] = nc.dram_tensor("x", [CIN, XTOT], F32, kind="ExternalInput").ap()
    for name, shape, dt_ in PARAMS:
        dram_dt = BF16 if dt_ is BF16 else F32
        di[name] = nc.dram_tensor(name, list(shape), dram_dt,
                                  kind="ExternalInput").ap()
    out_d = nc.dram_tensor("out", [COUT, NPIX], F32, kind="ExternalOutput").ap()

    with tile.TileContext(nc) as tc:
      # one long-lived pool holds every persistent tile (unique tag = own slot)
      with tc.tile_pool(name="perm", bufs=1) as perm:
        def ptile(name, shape, dt=F32):
            return perm.tile(list(shape), dt, name=name, tag=name)

        xk = [ptile(f"xk{kc}", [128, XTOT], F32R) for kc in range(2)]
        xr = ptile("xr", [128, NPIX], F32R)
        ctx_s = ptile("ctx_s", [128, NPIX], BF16)
        y_s = ptile("y_s", [96, Y_SZ], BF16)
        rowsum6 = ptile("rowsum6", [128, 576])
        S_s = ptile("S_s", [128, 64])

        sb = {}
        # critical-path first: conv weights, then the first x chunks, then
        # everything else
        sb["w1t"] = ptile("sb_w1t", [128, 2304], F32R)
        nc.sync.dma_start(out=sb["w1t"][:, :], in_=di["w1t"][:, :].bitcast(F32R))
        c0, c1 = XCHUNKS[0]
        for kc in range(2):
            nc.sync.dma_start(
                out=xk[kc][:, c0:c1],
                in_=di["x"][kc * 128:(kc + 1) * 128, c0:c1].bitcast(F32R))
        for name, shape, dt_ in PARAMS:
            if name == "w1t":
                continue
            sb[name] = ptile("sb_" + name, list(shape), dt_)
            src = di[name][:, :]
            if dt_ is F32R:
                src = src.bitcast(F32R)
            nc.sync.dma_start(out=sb[name][:, :], in_=src)
        for c0, c1 in XCHUNKS[1:]:
            for kc in range(2):
                nc.sync.dma_start(
                    out=xk[kc][:, c0:c1],
                    in_=di["x"][kc * 128:(kc + 1) * 128, c0:c1].bitcast(F32R))

        # one-time zero of the whole y tile: the 32-stride packing leaves
        # unused partitions whose wla2t rows are zero, but 0 * garbage-inf
        # would poison psum (engine APs need 32-aligned partition starts,
        # so zero everything once; per-rep guards are re-zeroed below)
        nc.gpsimd.memset(y_s[:, :], 0.0)

        for _rep in range(reps):
            # ============ Phase A: conv3x3 + BN + SiLU -> xr ==============
            nred = 0
            with tc.tile_pool(name="apsum", bufs=4, space="PSUM") as apool:
                for w in range(NWIN):
                    c0 = w * WIN
                    L = min(WIN, NPIX - c0)
                    ps = apool.tile([128, WIN], F32, tag="apsum")
                    for kc in range(2):
                        for dy in range(3):
                            for dx in range(3):
                                t = kc * 9 + dy * 3 + dx
                                off = c0 + dy * WP + dx
                                nc.tensor.matmul(
                                    ps[:, 0:L],
                                    sb["w1t"][:, t * 128:(t + 1) * 128],
                                    xk[kc][:, off:off + L],
                                    start=(t == 0), stop=(t == 17))
                    nc.scalar.activation(
                        xr[:, c0:c0 + L], ps[:, 0:L], AF.Silu,
                        bias=sb["bnbias"][:, 0:1])
                    # PSP stage 1 on finished 24-row chunks (rows r need cols
                    # through (r+1)*WP <= c0+L)
                    while nred < 4 and (nred + 1) * 24 * WP <= c0 + L:
                        r0 = nred * 24
                        xrb = xr[:, r0 * WP:(r0 + 24) * WP].rearrange(
                            "p (y c) -> p y c", c=WP)[:, :, 1:1 + W].rearrange(
                            "p y (j u) -> p y j u", u=16)
                        nc.vector.tensor_reduce(
                            rowsum6[:, nred * 144:(nred + 1) * 144].rearrange(
                                "p (y j) -> p y j", j=6),
                            xrb, axis=AX.X, op=ALU.add)
                        nred += 1

            # ============ Phase B: tokens + reformer gather ===============
            # S6 [128,36]: column sums of rowsum6 over 16-row groups
            nc.vector.tensor_reduce(
                S_s[:, 0:36].rearrange("p (i j) -> p i j", j=6),
                rowsum6[:, :].rearrange("p (i u j) -> p i j u", i=6, u=16, j=6),
                axis=AX.X, op=ALU.add)
            s3t = ptile("s3t", [128, 18])
            nc.vector.tensor_reduce(
                s3t[:, :].rearrange("p (i a j) -> p i a j", i=3, a=2, j=3),
                S_s[:, 0:36].rearrange("p (i a j b) -> p i a j b", i=3, a=2, j=3, b=2),
                axis=AX.X, op=ALU.add)
            nc.vector.tensor_reduce(
                S_s[:, 36:45].rearrange("p (i j) -> p i j", j=3),
                s3t[:, :].rearrange("p (i a j) -> p i j a", i=3, a=2, j=3),
                axis=AX.X, op=ALU.add)
            s2t = ptile("s2t", [128, 12])
            nc.vector.tensor_reduce(
                s2t[:, :].rearrange("p (i a j) -> p i a j", i=2, a=3, j=2),
                S_s[:, 0:36].rearrange("p (i a j b) -> p i a j b", i=2, a=3, j=2, b=3),
                axis=AX.X, op=ALU.add)
            nc.vector.tensor_reduce(
                S_s[:, 45:49].rearrange("p (i j) -> p i j", j=2),
                s2t[:, :].rearrange("p (i a j) -> p i j a", i=2, a=3, j=2),
                axis=AX.X, op=ALU.add)
            nc.vector.tensor_reduce(
                S_s[:, 49:50], S_s[:, 0:36].rearrange("p (i j) -> p i j", j=6),
                axis=AX.XY, op=ALU.add)

            kS = ptile("kS", [QD, NTOK])
            vS = ptile("vS", [128, NTOK])
            ktok8 = ptile("ktok8", [NH, QD])
            vtok8 = ptile("vtok8", [NH, 128])
            Lsc = ptile("Lsc", [NH, NTOK])
            eqt = ptile("eqt", [NH, NTOK])
            t1 = ptile("t1", [NH, NTOK])
            maxv = ptile("maxv", [NH, 1])
            bmin = ptile("bmin", [NH, 1])
            bT = ptile("bT", [1, NH])
            ranksrc = ptile("ranksrc", [NH, NH])
            eqm = ptile("eqm", [NH, NH])
            rank = ptile("rank", [NH, 1])
            onehot = ptile("onehot", [NH, NH])
            ksT = ptile("ksT", [QD, NH])
            vs_g = ptile("vs_g", [NH, 128], BF16)
            kwT = ptile("kwT", [128, NH], F32R)
            sbias = ptile("sbias", [NH, 1])

            with tc.tile_pool(name="bpsum", bufs=2, space="PSUM") as bpool:
                kS_p = bpool.tile([QD, NTOK], F32, tag="b1")
                nc.tensor.matmul(kS_p[:, :], sb["wkT"][:, :], S_s[:, 0:NTOK],
                                 start=True, stop=True)
                nc.scalar.copy(kS[:, :], kS_p[:, :])
                vS_p = bpool.tile([128, NTOK], F32, tag="b2")
                nc.tensor.matmul(vS_p[:, :], sb["wvT"][:, :], S_s[:, 0:NTOK],
                                 start=True, stop=True)
                nc.scalar.copy(vS[:, :], vS_p[:, :])

                # bucket logits over all 50 tokens (area-normalized + lsh@b_k)
                L_p = bpool.tile([NH, NTOK], F32, tag="b1")
                nc.tensor.matmul(L_p[:, :], sb["lshT"][:, :], kS[:, :],
                                 start=True, stop=True)
                nc.vector.tensor_tensor(Lsc[:, :], L_p[:, :], sb["invarea"][:, :],
                                        op=ALU.mult)
                nc.vector.tensor_scalar_add(Lsc[:, :], Lsc[:, :], sb["lshbk"][:, 0:1])
                # argmax over tokens (first occurrence)
                nc.vector.tensor_reduce(maxv[:, :], Lsc[:, :], axis=AX.X, op=ALU.max)
                nc.vector.tensor_scalar(eqt[:, :], Lsc[:, :], maxv[:, 0:1], None,
                                        op0=ALU.is_equal)
                # t1 = (eqt * -BIG) + (iota + BIG): iota where eq, BIG elsewhere
                nc.vector.scalar_tensor_tensor(
                    t1[:, :], eqt[:, :], -BIGF, sb["iotapb"][:, :],
                    op0=ALU.mult, op1=ALU.add)
                nc.vector.tensor_reduce(bmin[:, :], t1[:, :], axis=AX.X, op=ALU.min)

                # stable argsort rank of the 8 bucket ids
                bT_p = bpool.tile([1, NH], F32, tag="b1")
                nc.tensor.matmul(bT_p[:, :], bmin[:, :], sb["ident"][0:NH, 0:NH],
                                 start=True, stop=True)
                nc.scalar.copy(bT[:, :], bT_p[:, :])
                Bij_p = bpool.tile([NH, NH], F32, tag="b2")
                nc.tensor.matmul(Bij_p[:, :], sb["ones18"][:, :], bT[:, :],
                                 start=True, stop=True)
                nc.vector.tensor_scalar(ranksrc[:, :], Bij_p[:, :], bmin[:, 0:1], None,
                                        op0=ALU.is_lt)
                nc.vector.tensor_scalar(eqm[:, :], Bij_p[:, :], bmin[:, 0:1], None,
                                        op0=ALU.is_equal)
                nc.vector.tensor_tensor(eqm[:, :], eqm[:, :], sb["jlt"][:, :],
                                        op=ALU.mult)
                nc.vector.tensor_tensor(ranksrc[:, :], ranksrc[:, :], eqm[:, :],
                                        op=ALU.add)
                nc.vector.tensor_reduce(rank[:, :], ranksrc[:, :], axis=AX.X, op=ALU.add)
                nc.vector.tensor_scalar(onehot[:, :], sb["piota"][:, :], rank[:, 0:1],
                                        None, op0=ALU.is_equal)

                # first-8 tokens to [token, feat] layout (+ mean scale & bias)
                kt_p = bpool.tile([NH, QD], F32, tag="b1")
                nc.tensor.transpose(kt_p[:, :], kS[:, 0:NH], sb["ident"][0:QD, 0:QD])
                nc.vector.tensor_scalar(ktok8[:, :], kt_p[:, :], 1.0 / 256, None,
                                        op0=ALU.mult)
                nc.vector.tensor_tensor(ktok8[:, :], ktok8[:, :], sb["bk8"][:, :],
                                        op=ALU.add)
                vt_p = bpool.tile([NH, 128], F32, tag="b2")
                nc.tensor.transpose(vt_p[:, :], vS[:, 0:NH], sb["ident"][:, :])
                nc.vector.tensor_scalar(vtok8[:, :], vt_p[:, :], 1.0 / 256, None,
                                        op0=ALU.mult)
                nc.vector.tensor_tensor(vtok8[:, :], vtok8[:, :], sb["bv8"][:, :],
                                        op=ALU.add)

                # gather sorted tokens, fold w_q, score bias
                ksT_p = bpool.tile([QD, NH], F32, tag="b1")
                nc.tensor.matmul(ksT_p[:, :], ktok8[:, :], onehot[:, :],
                                 start=True, stop=True)
                nc.scalar.copy(ksT[:, :], ksT_p[:, :])
                vs_p = bpool.tile([NH, 128], F32, tag="b2")
                nc.tensor.matmul(vs_p[:, :], onehot[:, :], vtok8[:, :],
                                 start=True, stop=True)
                nc.scalar.copy(vs_g[:, :], vs_p[:, :])
                kw_p = bpool.tile([128, NH], F32, tag="b1")
                nc.tensor.matmul(kw_p[:, :], sb["wq"][:, :], ksT[:, :],
                                 start=True, stop=True)
                nc.scalar.copy(kwT[:, :], kw_p[:, :])
                sb_p = bpool.tile([NH, 1], F32, tag="b2")
                nc.tensor.matmul(sb_p[:, :], ksT[:, :], sb["bq"][:, :],
                                 start=True, stop=True)
                nc.scalar.copy(sbias[:, :], sb_p[:, :])

            # ======== Phase C1: attention (ACT runs exp only) =============
            with (
                tc.tile_pool(name="cpool", bufs=3) as cpool,
                tc.tile_pool(name="cps_s", bufs=2, space="PSUM") as ps_s,
                tc.tile_pool(name="cps_z", bufs=2, space="PSUM") as ps_z,
                tc.tile_pool(name="cps_ctx", bufs=2, space="PSUM") as ps_ctx,
            ):
                for w in range(NWIN):
                    c0 = w * WIN
                    L = min(WIN, NPIX - c0)
                    s_p = ps_s.tile([NH, WIN], F32, tag="s")
                    nc.tensor.matmul(s_p[:, 0:L], kwT[:, :], xr[:, c0:c0 + L],
                                     start=True, stop=True)
                    e_t = cpool.tile([NH, WIN], BF16, tag="e")
                    nc.scalar.activation(e_t[:, 0:L], s_p[:, 0:L], AF.Exp,
                                         bias=sbias[:, 0:1])
                    # z broadcast onto all 8 partitions via PE (a GPSIMD
                    # partition reduce measured ~4x slower than modeled), so
                    # the e-scale runs on SBUF bf16 with DVE's 2x packed mode
                    z_p = ps_z.tile([NH, WIN], F32, tag="z")
                    nc.tensor.matmul(z_p[:, 0:L], sb["ones88b"][:, :], e_t[:, 0:L],
                                     start=True, stop=True)
                    with nc.allow_low_precision(reason="bf16 softmax scale"):
                        rc_t = cpool.tile([NH, WIN], BF16, tag="rc")
                        nc.vector.reciprocal(rc_t[:, 0:L], z_p[:, 0:L])
                        e2_t = cpool.tile([NH, WIN], BF16, tag="e2")
                        nc.vector.tensor_tensor(e2_t[:, 0:L], e_t[:, 0:L],
                                                rc_t[:, 0:L], op=ALU.mult)
                    ctx_p = ps_ctx.tile([128, WIN], F32, tag="ctx")
                    nc.tensor.matmul(ctx_p[:, 0:L], vs_g[:, :], e2_t[:, 0:L],
                                     start=True, stop=True)
                    # psum -> SBUF move alternates ACT/DVE to balance load
                    if w % 2 == 0:
                        nc.scalar.copy(ctx_s[:, c0:c0 + L], ctx_p[:, 0:L])
                    else:
                        with nc.allow_low_precision(reason="bf16 ctx"):
                            nc.vector.tensor_copy(ctx_s[:, c0:c0 + L],
                                                  ctx_p[:, 0:L])

            # keep exp and silu in separate contiguous ACT runs (LUT reloads)
            tc.no_sync_barrier()

            # ==== Phase C2 (silu -> y group1 + Pool replicas) + Phase D ====
            def emit_y(t):
                c0 = t * TCOLS
                yp_p = ps_yp.tile([LA_MID, TCOLS], F32, tag="yp")
                nc.tensor.matmul(yp_p[:, :], sb["wla1T"][:, :],
                                 ctx_s[:, c0:c0 + TCOLS], start=True, stop=True)
                pv = yp_p[:, :].rearrange("p (r c) -> p r c", c=WP)[:, :, 1:1 + W]
                base = Y_G + c0
                dst1 = y_s[32:32 + LA_MID, base:base + TCOLS].rearrange(
                    "p (r c) -> p r c", c=WP)[:, :, 1:1 + W]
                nc.scalar.activation(dst1, pv, AF.Silu,
                                     bias=sb["labi"][:, 0:1],
                                     scale=sb["lasc"][:, 0:1])
                # shifted replicas for the dx taps: group0 = +1 col, group2 = -1
                # (split across the idle Pool engine and DVE's 2x bf16 path)
                for g, p0, eng in ((0, 0, nc.gpsimd), (2, 64, nc.vector)):
                    dstg = y_s[p0:p0 + LA_MID,
                               base + (1 - g):base + (1 - g) + TCOLS].rearrange(
                        "p (r c) -> p r c", c=WP)[:, :, 1:1 + W]
                    eng.tensor_copy(dstg, dst1)

            DG = 3      # tiles per consolidated DVE/DMA group
            grp = {}

            def emit_mask(t):
                c0 = t * TCOLS
                m_p = dps.tile([128, TCOLS], F32, tag="m")
                for dy in range(3):
                    off = Y_G + c0 + (dy - 1) * WP
                    nc.tensor.matmul(
                        m_p[:, :],
                        sb["wla2t"][:, dy * 128:(dy + 1) * 128],
                        y_s[:, off:off + TCOLS],
                        start=(dy == 0), stop=(dy == 2))
                gi = t % DG
                if gi == 0:
                    grp["msk"] = dpool.tile([128, DG * TCOLS], BF16, name="mskg", tag="msk")
                nc.scalar.activation(grp["msk"][:, gi * TCOLS:(gi + 1) * TCOLS],
                                     m_p[:, :], AF.Sigmoid)
                if gi == DG - 1:
                    g0 = (t - DG + 1) * TCOLS
                    GL = DG * TCOLS
                    ct = dpool.tile([128, GL], BF16, tag="ct")
                    with nc.allow_low_precision(reason="bf16 mask product"):
                        nc.vector.tensor_tensor(ct[:, :], ctx_s[:, g0:g0 + GL],
                                                grp["msk"][:, :], op=ALU.mult)
                    o_t = dpool.tile([128, GL], F32, tag="o")
                    nc.vector.tensor_tensor(o_t[:, :], ct[:, :],
                                            xr[:, g0:g0 + GL], op=ALU.add)
                    nc.sync.dma_start(out=out_d[:, g0:g0 + GL], in_=o_t[:, :])

            with (
                tc.tile_pool(name="dpool", bufs=2) as dpool,
                tc.tile_pool(name="cps_yp", bufs=2, space="PSUM") as ps_yp,
                tc.tile_pool(name="dpsum", bufs=2, space="PSUM") as dps,
            ):
                # two contiguous ACT runs (silu table, then sigmoid table):
                # interleaving emit_y/emit_mask would reload the LUT per tile
                for t in range(NT):
                    emit_y(t)
                for t in range(NT):
                    emit_mask(t)

    nc.compile()
    return nc


def get_program(reps=1):
    key = ("nc", reps)
    if key not in _BUILD_CACHE:
        _BUILD_CACHE[key] = build_program(reps)
    return _BUILD_CACHE[key]


def pad_x(xb):
    """[256,96,96] -> zero-framed [256, XTOT] fp32r-rounded."""
    xp = np.zeros((CIN, XTOT), np.float32)
    body = np.zeros((CIN, 98, WP), np.float32)
    body[:, 1:1 + H, 1:1 + W] = xb
    xp[:, 1:1 + 98 * WP] = body.reshape(CIN, 98 * WP)
    return _round_fp32r(xp)


def unpad_out(flat):
    """[128, 9408] padded rows -> [128, 96, 96]."""
    return np.ascontiguousarray(flat.reshape(COUT, H, WP)[:, :, 1:1 + W])


def kernel(**inputs):
    x = np.ascontiguousarray(np.asarray(inputs["x"], np.float32))
    assert x.shape == (B, CIN, H, W)
    weights = _host_prep(inputs)
    nc = get_program()
    in_maps = [dict(weights, x=pad_x(x[b])) for b in range(B)]
    res = run_bass_kernel_spmd(nc, in_maps, list(range(B)))
    out = np.stack([unpad_out(res.results[b]["out"]) for b in range(B)], axis=0)
    return out.astype(np.float32)
